# revision 12
# baseline (speedup 1.0000x reference)
"""Bass/Tile TRN2 kernel for nn_Attention_12704513261709.

8-way head-parallel attention: each of the 8 NeuronCores computes one head
(dh = 2048) over both batches, plus its partial (row-parallel) output
projection. Host sums the 8 partials.

Shapes (from reference.setup_inputs):
  x: (2, 2048, 256) f32, gamma: (256,), Wq/Wk/Wv: (16384, 256), Wo: (256, 16384)
"""

import numpy as np
import ml_dtypes

B = 2
N_SEQ = 2048
N_TOK = B * N_SEQ  # 4096
D = 256
HEADS = 8
INNER = 16384
DH = INNER // HEADS  # 2048
SCALE = 64 ** (-0.5)
EPS = 1e-5

FT = DH // 128  # 16 f-tiles per head dim
TT = N_SEQ // 128  # 16 key tiles per batch
NCH = N_SEQ // 512  # 4 query chunks of 512 per batch

_CACHE = {}


def _build():
    from concourse import bacc
    import concourse.tile as tile
    import concourse.mybir as mybir
    from concourse.masks import make_identity

    f32 = mybir.dt.float32
    bf16 = mybir.dt.bfloat16
    AF = mybir.ActivationFunctionType
    ALU = mybir.AluOpType

    nc = bacc.Bacc("TRN2", target_bir_lowering=False, debug=False, num_devices=8)

    x_d = nc.dram_tensor("x", [N_TOK, D], f32, kind="ExternalInput").ap()
    wqT_d = nc.dram_tensor("wqT", [D, DH], bf16, kind="ExternalInput").ap()
    wkT_d = nc.dram_tensor("wkT", [D, DH], bf16, kind="ExternalInput").ap()
    wvT_d = nc.dram_tensor("wvT", [D, DH], bf16, kind="ExternalInput").ap()
    woT_d = nc.dram_tensor("woT", [DH, D], bf16, kind="ExternalInput").ap()
    out_d = nc.dram_tensor("outT", [D, N_TOK], f32, kind="ExternalOutput").ap()

    with tile.TileContext(nc) as tc:
        with (
            tc.tile_pool(name="singles", bufs=1) as singles,
            tc.tile_pool(name="ln", bufs=3) as ln_pool,
            tc.tile_pool(name="big", bufs=1) as big,
            tc.tile_pool(name="qt", bufs=1) as qt_pool,
            tc.tile_pool(name="pt", bufs=1) as pt_pool,
            tc.tile_pool(name="ot", bufs=1) as ot_pool,
            tc.tile_pool(name="vstrip", bufs=5) as vs_pool,
            tc.tile_pool(name="stage", bufs=2) as stage_pool,
            tc.tile_pool(name="dram", bufs=2, space="DRAM") as dram_pool,
            tc.tile_pool(name="psA", bufs=3, space="PSUM") as psA,
            tc.tile_pool(name="psB", bufs=2, space="PSUM") as psB,
            tc.tile_pool(name="psR", bufs=1, space="PSUM") as psR,
            tc.tile_pool(name="psM", bufs=2, space="PSUM") as psM,
        ):
            identity = singles.tile([128, 128], f32)
            make_identity(nc, identity)
            ones = singles.tile([128, 1], bf16)
            nc.vector.memset(ones, 1.0)
            ones_row = singles.tile([1, 128], f32)
            nc.vector.memset(ones_row, 1.0)
            eps_t = singles.tile([128, 1], f32)
            nc.vector.memset(eps_t, EPS)
            warm = singles.tile([128, 1], f32)
            nc.scalar.activation(warm[:], eps_t[:], func=AF.Sqrt, bias=eps_t[:], scale=1.0)

            # weights to SBUF
            wqT = [big.tile([128, DH], bf16, tag=f"wq{d_}", name=f"wq{d_}") for d_ in range(2)]
            wkT = [big.tile([128, DH], bf16, tag=f"wk{d_}", name=f"wk{d_}") for d_ in range(2)]
            wvT = [big.tile([128, DH], bf16, tag=f"wv{d_}", name=f"wv{d_}") for d_ in range(2)]
            for d_ in range(2):
                sl = slice(d_ * 128, (d_ + 1) * 128)
                nc.gpsimd.dma_start(wvT[d_][:], wvT_d[sl, :])
                nc.gpsimd.dma_start(wkT[d_][:], wkT_d[sl, :])
                nc.gpsimd.dma_start(wqT[d_][:], wqT_d[sl, :])
            woT = [big.tile([128, D], bf16, tag=f"wo{fc}", name=f"wo{fc}") for fc in range(FT)]
            for fc in range(FT):
                nc.gpsimd.dma_start(woT[fc][:], woT_d[fc * 128 : (fc + 1) * 128, :])

            xnT = [big.tile([128, N_TOK], bf16, tag=f"xnT{d_}", name=f"xnT{d_}") for d_ in range(2)]

            state = {}

            def ln_tile(i):
                """LayerNorm token tile i (128 tokens), transpose into xnT."""
                x_t = ln_pool.tile([128, D], f32, tag="x", name="x")
                nc.gpsimd.dma_start(x_t[:], x_d[i * 128 : (i + 1) * 128, :])
                stats = ln_pool.tile([128, nc.vector.BN_STATS_DIM], f32, tag="st", name="st")
                nc.vector.bn_stats(stats[:], x_t[:])
                mv = ln_pool.tile([128, nc.vector.BN_AGGR_DIM], f32, tag="mv", name="mv")
                nc.vector.bn_aggr(mv[:], stats[:])
                std = ln_pool.tile([128, 1], f32, tag="std", name="std")
                nc.scalar.activation(
                    std[:], mv[:, 1:2], func=AF.Sqrt, bias=eps_t[:], scale=1.0
                )
                rstd = ln_pool.tile([128, 1], f32, tag="rstd", name="rstd")
                nc.vector.reciprocal(rstd[:], std[:])
                xn_t = ln_pool.tile([128, D], f32, tag="xn", name="xn")
                nc.vector.tensor_scalar(
                    xn_t[:],
                    x_t[:],
                    scalar1=mv[:, 0:1],
                    scalar2=rstd[:],
                    op0=ALU.subtract,
                    op1=ALU.mult,
                )
                for d_ in range(2):
                    ps = psM.tile([128, 512], f32, tag="m", name="m")
                    nc.tensor.transpose(
                        ps[:, :128], xn_t[:, d_ * 128 : (d_ + 1) * 128], identity[:]
                    )
                    nc.any.tensor_copy(xnT[d_][:, i * 128 : (i + 1) * 128], ps[:, :128])

            def kt_build_group(bb, nch):
                """K^T columns for one 512-token group of batch bb."""
                base = bb * N_SEQ
                for ft in range(FT):
                    ps = psM.tile([128, 512], f32, tag="m", name="m")
                    for d_ in range(2):
                        nc.tensor.matmul(
                            ps[:],
                            wkT[d_][:, ft * 128 : (ft + 1) * 128],
                            xnT[d_][:, base + nch * 512 : base + (nch + 1) * 512],
                            start=(d_ == 0),
                            stop=(d_ == 1),
                        )
                    nc.any.tensor_copy(
                        state[f"kt{ft}"][:, nch * 512 : (nch + 1) * 512], ps[:]
                    )

            def v_build_tile(bb, t):
                """V rows for key tile t of batch bb -> blocked DRAM scratch."""
                base = bb * N_SEQ
                v_stage = stage_pool.tile([128, DH], bf16, tag="vstage", name="vstage")
                for fch in range(4):
                    ps = psM.tile([128, 512], f32, tag="m", name="m")
                    for d_ in range(2):
                        nc.tensor.matmul(
                            ps[:],
                            xnT[d_][:, base + t * 128 : base + (t + 1) * 128],
                            wvT[d_][:, fch * 512 : (fch + 1) * 512],
                            start=(d_ == 0),
                            stop=(d_ == 1),
                        )
                    nc.any.tensor_copy(v_stage[:, fch * 512 : (fch + 1) * 512], ps[:])
                nc.sync.dma_start(
                    state["v_dram"][:, :, t, :].rearrange("g p f -> p g f"),
                    v_stage.rearrange("p (g f) -> p g f", g=FT),
                )

            def qt_build(bb, ch):
                cbase = bb * N_SEQ + ch * 512
                state["qt"] = []
                for ft in range(FT):
                    ps = psM.tile([128, 512], f32, tag="m", name="m")
                    for d_ in range(2):
                        nc.tensor.matmul(
                            ps[:],
                            wqT[d_][:, ft * 128 : (ft + 1) * 128],
                            xnT[d_][:, cbase : cbase + 512],
                            start=(d_ == 0),
                            stop=(d_ == 1),
                        )
                    qt = qt_pool.tile([128, 512], bf16, tag=f"qt{ft}", name=f"qt{ft}")
                    nc.any.tensor_copy(qt[:], ps[:])
                    state["qt"].append(qt)

            def phase_a(bb, ch):
                """S^T + exp per key tile; also prefetch V strips for phase B."""
                QT = state["qt"]
                KT = [state[f"kt{ft}"] for ft in range(FT)]
                state["pt"] = []
                state["strips"] = []
                for t in range(TT):
                    strip = vs_pool.tile([128, TT, 128], bf16, tag="vstrip", name="vstrip")
                    nc.sync.dma_start(strip[:], state["v_dram"][t])
                    state["strips"].append(strip)
                    st_ps = psA.tile([128, 512], f32, tag="st", name="st")
                    for ft in range(FT):
                        nc.tensor.matmul(
                            st_ps[:],
                            KT[ft][:, t * 128 : (t + 1) * 128],
                            QT[ft][:],
                            start=(ft == 0),
                            stop=(ft == FT - 1),
                        )
                    pt = pt_pool.tile([128, 512], bf16, tag=f"pt{t}", name=f"pt{t}")
                    nc.scalar.activation(pt[:], st_ps[:], func=AF.Exp)
                    state["pt"].append(pt)

            def phase_rs_mms():
                PT = state["pt"]
                rs_ps = psR.tile([1, 512], f32, tag="rs", name="rs")
                for t in range(TT):
                    nc.tensor.matmul(
                        rs_ps[:], ones[:], PT[t][:], start=(t == 0), stop=(t == TT - 1)
                    )
                recip = stage_pool.tile([1, 512], f32, tag="recip", name="recip")
                nc.vector.reciprocal(recip[:], rs_ps[:])
                state["recip"] = recip

            def phase_rs_finish():
                # emitted after phase B so the PE never waits on the DVE
                # reciprocal: by now it finished long ago
                rbc_ps = psM.tile([128, 512], f32, tag="m", name="rbc")
                nc.tensor.matmul(
                    rbc_ps[:], ones_row[:], state["recip"][:], start=True, stop=True
                )
                rbc_sb = stage_pool.tile([128, 512], f32, tag="rbc_sb", name="rbc_sb")
                nc.any.tensor_copy(rbc_sb[:], rbc_ps[:])
                state["rbc"] = rbc_sb

            def phase_b():
                PT = state["pt"]
                state["ot"] = []
                for fc in range(FT):
                    strip = state["strips"][fc]
                    ot_ps = psB.tile([128, 512], f32, tag="ot", name="ot")
                    for t in range(TT):
                        nc.tensor.matmul(
                            ot_ps[:],
                            strip[:, t, :],
                            PT[t][:],
                            start=(t == 0),
                            stop=(t == TT - 1),
                        )
                    ot = ot_pool.tile([128, 512], bf16, tag=f"ot{fc}", name=f"ot{fc}")
                    nc.any.tensor_copy(ot[:], ot_ps[:])
                    state["ot"].append(ot)

            def phase_c(bb, ch):
                cbase = bb * N_SEQ + ch * 512
                OT = state["ot"]
                for dm in range(2):
                    op_ps = psM.tile([128, 512], f32, tag="m", name="m")
                    for fc in range(FT):
                        nc.tensor.matmul(
                            op_ps[:],
                            woT[fc][:, dm * 128 : (dm + 1) * 128],
                            OT[fc][:],
                            start=(fc == 0),
                            stop=(fc == FT - 1),
                        )
                    op_sb = stage_pool.tile([128, 512], f32, tag="opsb", name="opsb")
                    nc.vector.tensor_tensor(
                        op_sb[:], op_ps[:], state["rbc"][:], ALU.mult
                    )
                    nc.sync.dma_start(
                        out_d[dm * 128 : (dm + 1) * 128, cbase : cbase + 512],
                        op_sb[:],
                    )

            def kv_alloc(bb):
                for ft in range(FT):
                    state[f"kt{ft}"] = big.tile(
                        [128, N_SEQ], bf16, tag=f"kt{ft}", name=f"kt{ft}"
                    )
                state["v_dram"] = dram_pool.tile(
                    [FT, 128, TT, 128], bf16, tag="vscratch", name="vscratch"
                )

            # ---- prologue: batch-0 LN interleaved with batch-0 K/V builds,
            # V/KT lagging one tile so transpose->copy latency stays hidden ----
            kv_alloc(0)
            for i in range(TT):
                ln_tile(i)
                if i > 0:
                    v_build_tile(0, i - 1)
                if i % 4 == 0 and i > 0:
                    kt_build_group(0, i // 4 - 1)
            v_build_tile(0, TT - 1)
            kt_build_group(0, 3)
            qt_build(0, 0)

            # ---- main loop over 8 chunks ----
            for bb, ch in [(b_, c_) for b_ in range(B) for c_ in range(NCH)]:
                phase_a(bb, ch)
                if bb == 0 and ch < NCH - 1:
                    # batch-1 layernorm, spread over batch-0 chunks 0..2 so the
                    # per-tile DVE chains never bunch up ahead of the transposes
                    for i in range(TT + 6 * ch, min(TT + 6 * (ch + 1), N_TOK // 128)):
                        ln_tile(i)
                # emit the next chunk's inputs right after A so the PE never
                # waits on the QT dependency chain at the chunk boundary
                if bb == 0 and ch == NCH - 1:
                    kv_alloc(1)
                    for nch in range(4):
                        kt_build_group(1, nch)
                    for t in range(TT):
                        v_build_tile(1, t)
                    qt_build(1, 0)
                elif ch < NCH - 1:
                    qt_build(bb, ch + 1)
                phase_rs_mms()
                phase_b()
                phase_rs_finish()
                phase_c(bb, ch)

    nc.compile()
    return nc


def get_nc():
    if "nc" not in _CACHE:
        _CACHE["nc"] = _build()
    return _CACHE["nc"]


def make_in_maps(x, gamma, Wq, Wk, Wv, Wo):
    bf = ml_dtypes.bfloat16
    gp = (1.0 + gamma.astype(np.float64))[None, :]
    x_flat = np.ascontiguousarray(x.reshape(N_TOK, D).astype(np.float32))
    in_maps = []
    for h in range(HEADS):
        sl = slice(h * DH, (h + 1) * DH)
        wq = (Wq[sl].astype(np.float64) * gp * SCALE).T.astype(bf)
        wk = (Wk[sl].astype(np.float64) * gp).T.astype(bf)
        wv = (Wv[sl].astype(np.float64) * gp).T.astype(bf)
        wo = Wo[:, sl].T.astype(bf)
        in_maps.append(
            {
                "x": x_flat,
                "wqT": np.ascontiguousarray(wq),
                "wkT": np.ascontiguousarray(wk),
                "wvT": np.ascontiguousarray(wv),
                "woT": np.ascontiguousarray(wo),
            }
        )
    return in_maps


def kernel(x, gamma, Wq, Wk, Wv, Wo):
    from concourse import bass_utils

    nc = get_nc()
    in_maps = make_in_maps(x, gamma, Wq, Wk, Wv, Wo)
    res = bass_utils.run_bass_kernel_spmd(
        nc, in_maps, core_ids=list(range(HEADS))
    )
    acc = np.zeros((D, N_TOK), np.float32)
    for h in range(HEADS):
        acc += res.results[h]["outT"]
    return np.ascontiguousarray(acc.T).reshape(B, N_SEQ, D).astype(np.float32)


# revision 13
# speedup vs baseline: 1.0313x; 1.0313x over previous
"""Bass/Tile TRN2 kernel for nn_Attention_12704513261709.

8-way head-parallel attention: each of the 8 NeuronCores computes one head
(dh = 2048) over both batches, plus its partial (row-parallel) output
projection. Host sums the 8 partials.

Shapes (from reference.setup_inputs):
  x: (2, 2048, 256) f32, gamma: (256,), Wq/Wk/Wv: (16384, 256), Wo: (256, 16384)
"""

import numpy as np
import ml_dtypes

B = 2
N_SEQ = 2048
N_TOK = B * N_SEQ  # 4096
D = 256
HEADS = 8
INNER = 16384
DH = INNER // HEADS  # 2048
SCALE = 64 ** (-0.5)
EPS = 1e-5

FT = DH // 128  # 16 f-tiles per head dim
TT = N_SEQ // 128  # 16 key tiles per batch
NCH = N_SEQ // 512  # 4 query chunks of 512 per batch

_CACHE = {}


def _build():
    from concourse import bacc
    import concourse.tile as tile
    import concourse.mybir as mybir
    from concourse.masks import make_identity

    f32 = mybir.dt.float32
    bf16 = mybir.dt.bfloat16
    AF = mybir.ActivationFunctionType
    ALU = mybir.AluOpType

    nc = bacc.Bacc("TRN2", target_bir_lowering=False, debug=False, num_devices=8)

    x_d = nc.dram_tensor("x", [N_TOK, D], f32, kind="ExternalInput").ap()
    wqT_d = nc.dram_tensor("wqT", [D, DH], bf16, kind="ExternalInput").ap()
    wkT_d = nc.dram_tensor("wkT", [D, DH], bf16, kind="ExternalInput").ap()
    wvT_d = nc.dram_tensor("wvT", [D, DH], bf16, kind="ExternalInput").ap()
    woT_d = nc.dram_tensor("woT", [DH, D], bf16, kind="ExternalInput").ap()
    out_d = nc.dram_tensor("outT", [D, N_TOK], f32, kind="ExternalOutput").ap()

    with tile.TileContext(nc) as tc:
        with (
            tc.tile_pool(name="singles", bufs=1) as singles,
            tc.tile_pool(name="ln", bufs=3) as ln_pool,
            tc.tile_pool(name="big", bufs=1) as big,
            tc.tile_pool(name="qt", bufs=1) as qt_pool,
            tc.tile_pool(name="pt", bufs=1) as pt_pool,
            tc.tile_pool(name="ot", bufs=1) as ot_pool,
            tc.tile_pool(name="vstrip", bufs=5) as vs_pool,
            tc.tile_pool(name="stage", bufs=2) as stage_pool,
            tc.tile_pool(name="dram", bufs=2, space="DRAM") as dram_pool,
            tc.tile_pool(name="psA", bufs=3, space="PSUM") as psA,
            tc.tile_pool(name="psB", bufs=2, space="PSUM") as psB,
            tc.tile_pool(name="psR", bufs=1, space="PSUM") as psR,
            tc.tile_pool(name="psM", bufs=2, space="PSUM") as psM,
        ):
            identity = singles.tile([128, 128], f32)
            make_identity(nc, identity)
            ones = singles.tile([128, 1], bf16)
            nc.vector.memset(ones, 1.0)
            ones_row = singles.tile([1, 128], f32)
            nc.vector.memset(ones_row, 1.0)
            eps_t = singles.tile([128, 1], f32)
            nc.vector.memset(eps_t, EPS)
            warm = singles.tile([128, 1], f32)
            nc.scalar.activation(warm[:], eps_t[:], func=AF.Sqrt, bias=eps_t[:], scale=1.0)

            # weights to SBUF
            wqT = [big.tile([128, DH], bf16, tag=f"wq{d_}", name=f"wq{d_}") for d_ in range(2)]
            wkT = [big.tile([128, DH], bf16, tag=f"wk{d_}", name=f"wk{d_}") for d_ in range(2)]
            wvT = [big.tile([128, DH], bf16, tag=f"wv{d_}", name=f"wv{d_}") for d_ in range(2)]
            for d_ in range(2):
                sl = slice(d_ * 128, (d_ + 1) * 128)
                nc.gpsimd.dma_start(wvT[d_][:], wvT_d[sl, :])
                nc.gpsimd.dma_start(wkT[d_][:], wkT_d[sl, :])
                nc.gpsimd.dma_start(wqT[d_][:], wqT_d[sl, :])
            woT = [big.tile([128, D], bf16, tag=f"wo{fc}", name=f"wo{fc}") for fc in range(FT)]
            for fc in range(FT):
                nc.gpsimd.dma_start(woT[fc][:], woT_d[fc * 128 : (fc + 1) * 128, :])

            xnT = [big.tile([128, N_TOK], bf16, tag=f"xnT{d_}", name=f"xnT{d_}") for d_ in range(2)]

            state = {}

            def ln_tile(i):
                """LayerNorm token tile i (128 tokens), transpose into xnT."""
                x_t = ln_pool.tile([128, D], f32, tag="x", name="x")
                nc.gpsimd.dma_start(x_t[:], x_d[i * 128 : (i + 1) * 128, :])
                stats = ln_pool.tile([128, nc.vector.BN_STATS_DIM], f32, tag="st", name="st")
                nc.vector.bn_stats(stats[:], x_t[:])
                mv = ln_pool.tile([128, nc.vector.BN_AGGR_DIM], f32, tag="mv", name="mv")
                nc.vector.bn_aggr(mv[:], stats[:])
                std = ln_pool.tile([128, 1], f32, tag="std", name="std")
                nc.scalar.activation(
                    std[:], mv[:, 1:2], func=AF.Sqrt, bias=eps_t[:], scale=1.0
                )
                rstd = ln_pool.tile([128, 1], f32, tag="rstd", name="rstd")
                nc.vector.reciprocal(rstd[:], std[:])
                xn_t = ln_pool.tile([128, D], f32, tag="xn", name="xn")
                nc.vector.tensor_scalar(
                    xn_t[:],
                    x_t[:],
                    scalar1=mv[:, 0:1],
                    scalar2=rstd[:],
                    op0=ALU.subtract,
                    op1=ALU.mult,
                )
                for d_ in range(2):
                    ps = psM.tile([128, 512], f32, tag="m", name="m")
                    nc.tensor.transpose(
                        ps[:, :128], xn_t[:, d_ * 128 : (d_ + 1) * 128], identity[:]
                    )
                    nc.any.tensor_copy(xnT[d_][:, i * 128 : (i + 1) * 128], ps[:, :128])

            def kt_build_group(bb, nch):
                """K^T columns for one 512-token group of batch bb."""
                base = bb * N_SEQ
                for ft in range(FT):
                    ps = psM.tile([128, 512], f32, tag="m", name="m")
                    for d_ in range(2):
                        nc.tensor.matmul(
                            ps[:],
                            wkT[d_][:, ft * 128 : (ft + 1) * 128],
                            xnT[d_][:, base + nch * 512 : base + (nch + 1) * 512],
                            start=(d_ == 0),
                            stop=(d_ == 1),
                        )
                    nc.any.tensor_copy(
                        state[f"kt{ft}"][:, nch * 512 : (nch + 1) * 512], ps[:]
                    )

            def v_build_tile(bb, t):
                """V rows for key tile t of batch bb -> blocked DRAM scratch."""
                base = bb * N_SEQ
                v_stage = stage_pool.tile([128, DH], bf16, tag="vstage", name="vstage")
                for fch in range(4):
                    ps = psM.tile([128, 512], f32, tag="m", name="m")
                    for d_ in range(2):
                        nc.tensor.matmul(
                            ps[:],
                            xnT[d_][:, base + t * 128 : base + (t + 1) * 128],
                            wvT[d_][:, fch * 512 : (fch + 1) * 512],
                            start=(d_ == 0),
                            stop=(d_ == 1),
                        )
                    nc.any.tensor_copy(v_stage[:, fch * 512 : (fch + 1) * 512], ps[:])
                nc.sync.dma_start(
                    state["v_dram"][:, :, t, :].rearrange("g p f -> p g f"),
                    v_stage.rearrange("p (g f) -> p g f", g=FT),
                )

            def qt_build(bb, ch):
                cbase = bb * N_SEQ + ch * 512
                state["qt"] = []
                for ft in range(FT):
                    ps = psA.tile([128, 512], f32, tag="st", name="qtps")
                    for d_ in range(2):
                        nc.tensor.matmul(
                            ps[:],
                            wqT[d_][:, ft * 128 : (ft + 1) * 128],
                            xnT[d_][:, cbase : cbase + 512],
                            start=(d_ == 0),
                            stop=(d_ == 1),
                        )
                    qt = qt_pool.tile([128, 512], bf16, tag=f"qt{ft}", name=f"qt{ft}")
                    nc.any.tensor_copy(qt[:], ps[:])
                    state["qt"].append(qt)

            def phase_a(bb, ch):
                """S^T + exp per key tile; also prefetch V strips for phase B."""
                QT = state["qt"]
                KT = [state[f"kt{ft}"] for ft in range(FT)]
                state["pt"] = []
                state["strips"] = []
                for t in range(TT):
                    strip = vs_pool.tile([128, TT, 128], bf16, tag="vstrip", name="vstrip")
                    nc.gpsimd.dma_start(strip[:], state["v_dram"][t])
                    state["strips"].append(strip)
                    st_ps = psA.tile([128, 512], f32, tag="st", name="st")
                    for ft in range(FT):
                        nc.tensor.matmul(
                            st_ps[:],
                            KT[ft][:, t * 128 : (t + 1) * 128],
                            QT[ft][:],
                            start=(ft == 0),
                            stop=(ft == FT - 1),
                        )
                    pt = pt_pool.tile([128, 512], bf16, tag=f"pt{t}", name=f"pt{t}")
                    nc.scalar.activation(pt[:], st_ps[:], func=AF.Exp)
                    state["pt"].append(pt)

            def phase_rs_mms():
                PT = state["pt"]
                rs_ps = psR.tile([1, 512], f32, tag="rs", name="rs")
                for t in range(TT):
                    nc.tensor.matmul(
                        rs_ps[:], ones[:], PT[t][:], start=(t == 0), stop=(t == TT - 1)
                    )
                recip = stage_pool.tile([1, 512], f32, tag="recip", name="recip")
                nc.vector.reciprocal(recip[:], rs_ps[:])
                state["recip"] = recip

            def phase_rs_finish():
                # emitted after phase B so the PE never waits on the DVE
                # reciprocal: by now it finished long ago
                rbc_ps = psM.tile([128, 512], f32, tag="m", name="rbc")
                nc.tensor.matmul(
                    rbc_ps[:], ones_row[:], state["recip"][:], start=True, stop=True
                )
                rbc_sb = stage_pool.tile([128, 512], f32, tag="rbc_sb", name="rbc_sb")
                nc.any.tensor_copy(rbc_sb[:], rbc_ps[:])
                state["rbc"] = rbc_sb

            def phase_b():
                PT = state["pt"]
                state["ot"] = []
                for fc in range(FT):
                    strip = state["strips"][fc]
                    ot_ps = psB.tile([128, 512], f32, tag="ot", name="ot")
                    for t in range(TT):
                        nc.tensor.matmul(
                            ot_ps[:],
                            strip[:, t, :],
                            PT[t][:],
                            start=(t == 0),
                            stop=(t == TT - 1),
                        )
                    ot = ot_pool.tile([128, 512], bf16, tag=f"ot{fc}", name=f"ot{fc}")
                    nc.any.tensor_copy(ot[:], ot_ps[:])
                    state["ot"].append(ot)

            def phase_c(bb, ch):
                cbase = bb * N_SEQ + ch * 512
                OT = state["ot"]
                for dm in range(2):
                    op_ps = psM.tile([128, 512], f32, tag="m", name="m")
                    for fc in range(FT):
                        nc.tensor.matmul(
                            op_ps[:],
                            woT[fc][:, dm * 128 : (dm + 1) * 128],
                            OT[fc][:],
                            start=(fc == 0),
                            stop=(fc == FT - 1),
                        )
                    op_sb = stage_pool.tile([128, 512], f32, tag="opsb", name="opsb")
                    nc.vector.tensor_tensor(
                        op_sb[:], op_ps[:], state["rbc"][:], ALU.mult
                    )
                    nc.sync.dma_start(
                        out_d[dm * 128 : (dm + 1) * 128, cbase : cbase + 512],
                        op_sb[:],
                    )

            def kv_alloc(bb):
                for ft in range(FT):
                    state[f"kt{ft}"] = big.tile(
                        [128, N_SEQ], bf16, tag=f"kt{ft}", name=f"kt{ft}"
                    )
                state["v_dram"] = dram_pool.tile(
                    [FT, 128, TT, 128], bf16, tag="vscratch", name="vscratch"
                )

            # ---- prologue: batch-0 LN interleaved with batch-0 K/V builds,
            # V/KT lagging one tile so transpose->copy latency stays hidden ----
            kv_alloc(0)
            for i in range(TT):
                ln_tile(i)
                if i > 0:
                    v_build_tile(0, i - 1)
                if i % 4 == 0 and i > 0:
                    kt_build_group(0, i // 4 - 1)
            v_build_tile(0, TT - 1)
            kt_build_group(0, 3)
            qt_build(0, 0)

            # ---- main loop over 8 chunks ----
            for bb, ch in [(b_, c_) for b_ in range(B) for c_ in range(NCH)]:
                phase_a(bb, ch)
                if bb == 0 and ch < NCH - 1:
                    # batch-1 layernorm, spread over batch-0 chunks 0..2 so the
                    # per-tile DVE chains never bunch up ahead of the transposes
                    for i in range(TT + 6 * ch, min(TT + 6 * (ch + 1), N_TOK // 128)):
                        ln_tile(i)
                # emit the next chunk's inputs right after A so the PE never
                # waits on the QT dependency chain at the chunk boundary
                if bb == 0 and ch == NCH - 1:
                    kv_alloc(1)
                    for nch in range(4):
                        kt_build_group(1, nch)
                    for t in range(TT):
                        v_build_tile(1, t)
                    qt_build(1, 0)
                elif ch < NCH - 1:
                    qt_build(bb, ch + 1)
                phase_rs_mms()
                phase_b()
                phase_rs_finish()
                phase_c(bb, ch)

    nc.compile()
    return nc


def get_nc():
    if "nc" not in _CACHE:
        _CACHE["nc"] = _build()
    return _CACHE["nc"]


def make_in_maps(x, gamma, Wq, Wk, Wv, Wo):
    bf = ml_dtypes.bfloat16
    gp = (1.0 + gamma.astype(np.float64))[None, :]
    x_flat = np.ascontiguousarray(x.reshape(N_TOK, D).astype(np.float32))
    in_maps = []
    for h in range(HEADS):
        sl = slice(h * DH, (h + 1) * DH)
        wq = (Wq[sl].astype(np.float64) * gp * SCALE).T.astype(bf)
        wk = (Wk[sl].astype(np.float64) * gp).T.astype(bf)
        wv = (Wv[sl].astype(np.float64) * gp).T.astype(bf)
        wo = Wo[:, sl].T.astype(bf)
        in_maps.append(
            {
                "x": x_flat,
                "wqT": np.ascontiguousarray(wq),
                "wkT": np.ascontiguousarray(wk),
                "wvT": np.ascontiguousarray(wv),
                "woT": np.ascontiguousarray(wo),
            }
        )
    return in_maps


def kernel(x, gamma, Wq, Wk, Wv, Wo):
    from concourse import bass_utils

    nc = get_nc()
    in_maps = make_in_maps(x, gamma, Wq, Wk, Wv, Wo)
    res = bass_utils.run_bass_kernel_spmd(
        nc, in_maps, core_ids=list(range(HEADS))
    )
    acc = np.zeros((D, N_TOK), np.float32)
    for h in range(HEADS):
        acc += res.results[h]["outT"]
    return np.ascontiguousarray(acc.T).reshape(B, N_SEQ, D).astype(np.float32)
